# revision 9
# baseline (speedup 1.0000x reference)
"""Trainium2 Bass kernel for nn_CCELoss (calibration-histogram loss).

Sharding: data-parallel over image rows, 8 NeuronCores, 128 rows each.

Per-core layout: logits as [114 = 6 pixel-groups x 19 classes, F=45056]
(group g covers core-flat pixels [g*F, (g+1)*F); tail of group 5 is padding
with logits (+40, -40 x 18) -> p = (1.0, 0...) exactly in bf16, corrected
exactly on host).

Three super-blocks of 4/4/3 tiles to minimize ACT table switches (exp and
ln live in different activation-table sets; only copy/relu/sign are in
both):
  Phase A (per tile): ACT e = exp(l) bf16; PE Z via 16 accumulated one-hot
    block matmuls per half -> PSUM [96 = (g, qq128), 256]; ACT copy -> Zbuf
    [96, 2816] (copy needs no table switch).
  Phase B (per super-block): ACT m = ln(Z) in place (one table switch);
    DMA m to host; ACT m_hi = bf16(m); GPSIMD m_lo = bf16(m - m_hi).
  Phase C (per tile): DMA m_hi/m_lo -> [6, .]; PE d = -bc@m_hi - bc@m_lo
    + I@l (fp32r identity, accumulated in PSUM); ACT p = exp(d) bf16 with
    accum_out = sum p (one table switch back); folds.
Folds over bf16 p: DVE 4x-mode tensor_scalar for N1..N7, R1..R8
(is_gt/max-sub with fused accumulate), ACT Relu+accum for R9, GPSIMD for
N8, N9.  All data DMAs issue from the SP sequencer (HWDGE) so the Pool
engine only runs its folds.

The host computes the true-class p* = exp(l* - m) from the returned m,
bins it against target for the accuracy histogram, removes the padding
contribution exactly, and applies the loss formula.
"""

import numpy as np

import bass_rust
import concourse.bass as bass
from concourse import bacc
import concourse.mybir as mybir
import concourse.tile as tile
from concourse.vector_clock import ScopedClock
from concourse.bass_utils import run_bass_kernel_spmd

F32 = mybir.dt.float32
F32R = mybir.dt.float32r
BF16 = mybir.dt.bfloat16
AF = mybir.ActivationFunctionType
ALU = mybir.AluOpType

# ---------------- problem geometry (hardcoded) ----------------
C = 19
NB = 10
H, W = 1024, 2048
NCORES = 8
ROWS = H // NCORES          # 128
NPIX = ROWS * W             # 262144 valid pixels per core
G = 6
P = G * C                   # 114 partitions
TILE_F = 4096
NT = 11
F = NT * TILE_F             # 45056
NPAD = G * F - NPIX         # 8192 pad pixels
VALID_J5 = NPIX - 5 * F     # 36864 valid pixels in group 5
PAD_TILE0 = VALID_J5 // TILE_F  # = 9; tiles 9,10 have group 5 all-pad

THR = [np.float32(i / 10.0) for i in range(10)]
PADLO, PADHI = -40.0, 40.0

# fold slot layout per tile: N1..N9 at 0..8, R1..R9 at 9..17,
# R0 (ACT exp-accum chunks) at 18..21.  Engine assignment per (tile, fold)
# balances DVE (4x, cheapest) vs ACT vs GPSIMD.
NSLOT = 22
SL_N = 0
SL_R = 9
SL_R0 = 18
ACT_R8_TILES = {0, 3, 6, 9}                  # R8 on ACT for these tiles
GP_N7_TILES = {1, 2, 4, 5, 7, 8, 10}         # N7 on GPSIMD for these tiles

MM_CHUNK = 512
PCOLS = NT * 256            # m cols: (t, h, c128) -> t*256 + h*128 + c
SBS = [(0, 2), (2, 5), (5, 8), (8, NT)]     # super-blocks

_BUILD_CACHE = {}


def _patch_tile_drain():
    """walrus rejects drains with >1 sync wait; split the tile-exit drain."""
    if getattr(tile.TileContext, "_drain_patched", False):
        return

    def _drain_and_barrier(self, tick_clock, wait_clock):
        drain_inst = self.nc.sync.drain()
        wait_clock.add_sem_waits(
            drain_inst.ins, ScopedClock({None: tick_clock.global_clock})
        )
        si = drain_inst.ins.sync_info
        if si is not None and len(si.on_wait) > 1:
            waits = list(si.on_wait)
            ups = list(si.on_update)
            drain_inst.ins.sync_info = mybir.SyncInfo(on_wait=waits[:1], on_update=[])
            last = drain_inst
            for i in range(1, len(waits)):
                last = self.nc.sync.drain()
                last.ins.sync_info = mybir.SyncInfo(on_wait=waits[i:i + 1], on_update=[])
            if ups:
                lw = list(last.ins.sync_info.on_wait) if last.ins.sync_info else []
                last.ins.sync_info = mybir.SyncInfo(on_wait=lw, on_update=ups)
        self.nc.all_engine_barrier()
        assert self.sems is not None
        popped = self.nc._tile_sem_poison_stack.pop()
        assert popped is self._sem_poison
        self.nc.clear_and_free_semaphores(list(self.sems.allocated().values()))
        self.nc.all_engine_barrier()

    tile.TileContext._drain_and_barrier = _drain_and_barrier
    tile.TileContext._drain_patched = True


def build_nc():
    _patch_tile_drain()
    nc = bacc.Bacc()

    # register threshold constants (ACT bias operands) as const APs
    for v in (float(-THR[8]), float(-THR[9])):
        if (F32, v) not in nc.const_aps.aps:
            tns = nc.alloc_sbuf_tensor(f"const-thr-{v}", [128, 1], F32)
            nc.gpsimd.memset(tns.ap(), v)
            nc.const_aps.aps[(F32, v)] = tns.ap()
    nc.all_engine_barrier()

    lg = nc.declare_dram_parameter("lg", [C, NPIX], F32, isOutput=False)
    zpad = nc.declare_dram_parameter("zpad", [C, TILE_F], F32, isOutput=False)
    bdq = nc.declare_dram_parameter("bdq", [P, 16 * 96], BF16, isOutput=False)
    bcneg = nc.declare_dram_parameter("bcneg", [G, P], BF16, isOutput=False)
    ident = nc.declare_dram_parameter("ident", [P, P], F32, isOutput=False)
    folds_out = nc.declare_dram_parameter("folds", [P, NT * NSLOT], F32, isOutput=True)
    m_out = nc.declare_dram_parameter("mlog", [96, PCOLS], F32, isOutput=True)

    with tile.TileContext(nc) as tc:
        with (
            tc.tile_pool(name="const", bufs=1) as constp,
            tc.tile_pool(name="lt", bufs=5) as lp,
            tc.tile_pool(name="et", bufs=2) as ep,
            tc.tile_pool(name="pt", bufs=3) as pp,
            tc.tile_pool(name="m6", bufs=2) as mp,
            tc.tile_pool(name="acc", bufs=1) as accp,
            tc.tile_pool(name="zpsum", bufs=2, space="PSUM") as zp,
            tc.tile_pool(name="dpsum", bufs=3, space="PSUM") as dp,
        ):
            bdq_sb = constp.tile([P, 16 * 96], BF16)
            nc.sync.dma_start(out=bdq_sb[:], in_=bdq[:])
            bc_sb = constp.tile([G, P], BF16)
            nc.sync.dma_start(out=bc_sb[:], in_=bcneg[:])
            id_sb = constp.tile([P, P], F32)
            nc.sync.dma_start(out=id_sb[:], in_=ident[:])

            foldacc = accp.tile([P, NT * NSLOT], F32)
            zbuf = accp.tile([96, PCOLS], F32)    # Z, then ln(Z) in place
            mhi = accp.tile([96, PCOLS], BF16)
            mlo = accp.tile([96, PCOLS], BF16)
            scr_dve = accp.tile([P, TILE_F], BF16)
            scr_act = accp.tile([P, TILE_F], BF16)
            scr_gp = accp.tile([P, TILE_F], BF16)

            lts = {}
            for sb0, sb1 in SBS:
                # ---- phase A: load, exp, Z ----
                for t in range(sb0, sb1):
                    lt = lp.tile([P, TILE_F], F32)
                    lts[t] = lt
                    ng = G if t < PAD_TILE0 else G - 1
                    base = lg[:, t * TILE_F:(t + 1) * TILE_F]
                    src3 = bass_rust.AP(
                        tensor=base.tensor, offset=base.offset,
                        ap=[[F, ng]] + list(base.ap))
                    nc.sync.dma_start(out=lt[0:C * ng, :], in_=src3)
                    if ng < G:
                        nc.sync.dma_start(out=lt[C * 5:P, :], in_=zpad[:])

                    et = ep.tile([P, TILE_F], BF16)
                    nc.scalar.activation(et[:], lt[:], AF.Exp)

                    # Z[(g, qq), h*128 + c] for the whole tile in one PSUM
                    # tile [96, 256]; 16 accumulated matmuls per half
                    zps = zp.tile([96, 256], F32)
                    for h in range(2):
                        for qq in range(16):
                            c0 = h * 2048 + qq * 128
                            nc.tensor.matmul(
                                zps[:, h * 128:(h + 1) * 128],
                                bdq_sb[:, qq * 96:(qq + 1) * 96],
                                et[:, c0:c0 + 128],
                                start=(qq == 0), stop=(qq == 15),
                            )
                    nc.scalar.activation(
                        zbuf[:, t * 256:(t + 1) * 256], zps[:], AF.Copy,
                    )

                # ---- phase B: ln + split (one table switch to ln set) ----
                sl = slice(sb0 * 256, sb1 * 256)
                nc.scalar.activation(zbuf[:, sl], zbuf[:, sl], AF.Ln)
                nc.sync.dma_start(out=m_out[:, sl], in_=zbuf[:, sl])
                nc.scalar.activation(mhi[:, sl], zbuf[:, sl], AF.Copy)
                nc.gpsimd.tensor_sub(mlo[:, sl], zbuf[:, sl], mhi[:, sl])

                # ---- phase C: d matmuls, exp(d), folds (switch back to exp) --
                for t in range(sb0, sb1):
                    lt = lts.pop(t)
                    # m6: [6, 2*4096] bf16; col = s*4096 + qq*256 + h*128 + c
                    # (row-major unfold of mhi/mlo rows; the matmul moving
                    # operand for a (qt, h) 512-chunk is the strided AP
                    # [4 x 128 @ stride 256] at offset qt*1024 + h*128)
                    m6 = mp.tile([G, 2 * TILE_F], BF16)
                    msl = slice(t * 256, (t + 1) * 256)
                    nc.sync.dma_start(out=m6[:, 0:TILE_F], in_=mhi[:, msl])
                    nc.sync.dma_start(out=m6[:, TILE_F:2 * TILE_F],
                                      in_=mlo[:, msl])
                    m6ap = m6[:]

                    def m6_mov(off):
                        return bass_rust.AP(
                            tensor=m6ap.tensor, offset=m6ap.offset + off,
                            ap=[list(m6ap.ap[0]), [256, 4], [1, 128]])

                    pt = pp.tile([P, TILE_F], BF16)
                    base_sl = t * NSLOT
                    for hb in range(4):      # four [114, 1024] PSUM blocks
                        dps = dp.tile([P, 1024], F32)
                        for q in range(2):
                            n0 = hb * 1024 + q * MM_CHUNK
                            h, qt = n0 // 2048, (n0 % 2048) // MM_CHUNK
                            mcol = qt * 1024 + h * 128
                            out_sl = dps[:, q * MM_CHUNK:(q + 1) * MM_CHUNK]
                            nc.tensor.matmul(
                                out_sl, bc_sb[:], m6_mov(mcol),
                                start=True, stop=False,
                            )
                            nc.tensor.matmul(
                                out_sl, bc_sb[:], m6_mov(TILE_F + mcol),
                                start=False, stop=False,
                            )
                            nc.tensor.matmul(
                                out_sl, id_sb[:].bitcast(F32R),
                                lt[:, n0:n0 + MM_CHUNK].bitcast(F32R),
                                start=False, stop=True,
                            )
                        nc.scalar.activation(
                            pt[:, hb * 1024:(hb + 1) * 1024], dps[:], AF.Exp,
                            accum_out=foldacc[:, base_sl + SL_R0 + hb:
                                              base_sl + SL_R0 + hb + 1],
                        )

                    # ---- folds over bf16 p ----
                    gp_counts = [8, 9] + ([7] if t in GP_N7_TILES else [])
                    act_confs = [9] + ([8] if t in ACT_R8_TILES else [])
                    for i in range(1, 10):   # counts
                        col = foldacc[:, base_sl + SL_N + i - 1:
                                      base_sl + SL_N + i]
                        if i in gp_counts:
                            nc.gpsimd.tensor_scalar(
                                scr_gp[:], pt[:], float(THR[i]), None,
                                ALU.is_gt, ALU.add, accum_out=col,
                            )
                        else:
                            nc.vector.tensor_scalar(
                                scr_dve[:], pt[:], float(THR[i]), None,
                                ALU.is_gt, ALU.add, accum_out=col,
                            )
                    for i in range(1, 10):   # conf sums
                        col = foldacc[:, base_sl + SL_R + i - 1:
                                      base_sl + SL_R + i]
                        if i in act_confs:
                            nc.scalar.activation(
                                scr_act[:], pt[:], AF.Relu,
                                bias=-float(THR[i]), accum_out=col,
                            )
                        else:
                            nc.vector.tensor_scalar(
                                scr_dve[:], pt[:], float(THR[i]),
                                float(THR[i]), ALU.max, ALU.subtract,
                                accum_out=col,
                            )

            # ---- end phase ----
            nc.sync.dma_start(out=folds_out[:], in_=foldacc[:])

    nc.finalize()
    return nc


def _make_consts():
    # bdq: 16 stationaries [114, 96]; block qq maps class-group g of
    # moving chunk qq*128 to output row (g, qq)
    bdq = np.zeros((P, 16 * 96), np.float32)
    for qq in range(16):
        for g in range(G):
            bdq[C * g:C * (g + 1), qq * 96 + g * 16 + qq] = 1.0
    bc = np.zeros((G, P), np.float32)
    for g in range(G):
        bc[g, C * g:C * (g + 1)] = -1.0
    return bdq, bc


def _shard_host(output: np.ndarray, target: np.ndarray):
    o = np.ascontiguousarray(output[0])          # [19, 1024, 2048]
    bdq, bc = _make_consts()
    zp = np.full((C, TILE_F), PADLO, np.float32)
    zp[0, :] = PADHI

    in_maps = []
    for core in range(NCORES):
        r0 = core * ROWS
        lgc = np.ascontiguousarray(o[:, r0:r0 + ROWS, :].reshape(C, NPIX))
        in_maps.append({
            "lg": lgc, "bdq": bdq, "bcneg": bc,
            "ident": np.eye(P, dtype=np.float32),
            "zpad": zp,
        })
    return in_maps


def _m_to_flat(m: np.ndarray) -> np.ndarray:
    """[96, PCOLS] (g,qq)-major -> core-flat [G*F] (incl pad)."""
    return (m.reshape(G, 16, NT, 2, 128).transpose(0, 2, 3, 1, 4).reshape(-1))


def _decode_and_loss(results, output: np.ndarray, target: np.ndarray):
    conf = np.zeros((C, NB), np.float64)
    cnt = np.zeros((C, NB), np.float64)
    acc = np.zeros((C, NB), np.float64)

    o = output[0]
    lstar_full = np.take_along_axis(o, target[0][None].astype(np.int64), axis=0)[0]

    for core in range(NCORES):
        folds = results[core]["folds"].astype(np.float64)
        folds = folds.reshape(P, NT, NSLOT).sum(axis=1)      # [114, 22]
        folds = folds.reshape(G, C, NSLOT).sum(axis=0)       # [C, 22]
        Ni = np.concatenate(
            [np.full((C, 1), float(NPIX)),
             folds[:, SL_N:SL_N + 9]], axis=1)               # [C, 10]
        R = np.concatenate(
            [folds[:, SL_R0:SL_R0 + 4].sum(axis=1, keepdims=True),
             folds[:, SL_R:SL_R + 9]], axis=1)               # [C, 10]

        # pad pixels land on class 0 with p = 1.0 exactly: remove them
        Ni[0, 1:] -= NPAD
        tgrid = np.arange(10, dtype=np.float64) / 10.0
        R[0, :] -= NPAD * (1.0 - tgrid)

        S = R + tgrid[None, :] * Ni              # S_i = sum p * [p > t_i]
        Snext = np.concatenate([S[:, 1:], np.zeros((C, 1))], axis=1)
        Nnext = np.concatenate([Ni[:, 1:], np.zeros((C, 1))], axis=1)
        conf += S - Snext
        cnt += Ni - Nnext

        # accuracy histogram from host-side p* = exp(l* - m)
        r0 = core * ROWS
        m = _m_to_flat(results[core]["mlog"])[:NPIX]
        ls = lstar_full[r0:r0 + ROWS, :].reshape(-1)
        ps = np.exp(ls - m).astype(np.float32)
        y = target[0, r0:r0 + ROWS, :].reshape(-1)
        b = np.clip(np.ceil(ps * np.float32(10.0)).astype(np.int32) - 1, 0, NB - 1)
        acc += np.bincount(y * NB + b, minlength=C * NB).reshape(C, NB)

    EPS = 1e-13
    avg_acc = acc / (cnt + EPS)
    avg_conf = conf / (cnt + EPS)
    loss = np.sum((avg_acc - avg_conf) ** 2 * (cnt / cnt.sum()))
    return np.float32(loss), (conf, cnt, acc)


def kernel(output: np.ndarray, target: np.ndarray) -> np.ndarray:
    output = np.asarray(output, np.float32)
    target = np.asarray(target, np.int32)
    if "nc" not in _BUILD_CACHE:
        _BUILD_CACHE["nc"] = build_nc()
    nc = _BUILD_CACHE["nc"]
    in_maps = _shard_host(output, target)
    res = run_bass_kernel_spmd(nc, in_maps, list(range(NCORES)))
    loss, _ = _decode_and_loss(res.results, output, target)
    return np.float32(loss)


# revision 10
# speedup vs baseline: 1.0975x; 1.0975x over previous
"""Trainium2 Bass kernel for nn_CCELoss (calibration-histogram loss).

Sharding: data-parallel over image rows, 8 NeuronCores, 128 rows each.

Per-core layout: logits as [114 = 6 pixel-groups x 19 classes, F=45056]
(group g covers core-flat pixels [g*F, (g+1)*F); tail of group 5 is padding
with logits (+40, -40 x 18) -> p = (1.0, 0...) exactly in bf16, corrected
exactly on host).

Three super-blocks of 4/4/3 tiles to minimize ACT table switches (exp and
ln live in different activation-table sets; only copy/relu/sign are in
both):
  Phase A (per tile): ACT e = exp(l) bf16; PE Z via 16 accumulated one-hot
    block matmuls per half -> PSUM [96 = (g, qq128), 256]; ACT copy -> Zbuf
    [96, 2816] (copy needs no table switch).
  Phase B (per super-block): ACT m = ln(Z) in place (one table switch);
    DMA m to host; ACT m_hi = bf16(m); GPSIMD m_lo = bf16(m - m_hi).
  Phase C (per tile): DMA m_hi/m_lo -> [6, .]; PE d = -bc@m_hi - bc@m_lo
    + I@l (fp32r identity, accumulated in PSUM); ACT p = exp(d) bf16 with
    accum_out = sum p (one table switch back); folds.
Folds over bf16 p: DVE 4x-mode tensor_scalar for N1..N7, R1..R8
(is_gt/max-sub with fused accumulate), ACT Relu+accum for R9, GPSIMD for
N8, N9.  All data DMAs issue from the SP sequencer (HWDGE) so the Pool
engine only runs its folds.

The host computes the true-class p* = exp(l* - m) from the returned m,
bins it against target for the accuracy histogram, removes the padding
contribution exactly, and applies the loss formula.
"""

import numpy as np

import bass_rust
import concourse.bass as bass
from concourse import bacc
import concourse.mybir as mybir
import concourse.tile as tile
from concourse.vector_clock import ScopedClock
from concourse.bass_utils import run_bass_kernel_spmd

F32 = mybir.dt.float32
F32R = mybir.dt.float32r
BF16 = mybir.dt.bfloat16
AF = mybir.ActivationFunctionType
ALU = mybir.AluOpType

# ---------------- problem geometry (hardcoded) ----------------
C = 19
NB = 10
H, W = 1024, 2048
NCORES = 8
ROWS = H // NCORES          # 128
NPIX = ROWS * W             # 262144 valid pixels per core
G = 6
P = G * C                   # 114 partitions
TILE_F = 4096
NT = 11
F = NT * TILE_F             # 45056
NPAD = G * F - NPIX         # 8192 pad pixels
VALID_J5 = NPIX - 5 * F     # 36864 valid pixels in group 5
PAD_TILE0 = VALID_J5 // TILE_F  # = 9; tiles 9,10 have group 5 all-pad

THR = [np.float32(i / 10.0) for i in range(10)]
PADLO, PADHI = -40.0, 40.0

# fold slot layout per tile: N1..N9 at 0..8, R1..R9 at 9..17,
# R0 (ACT exp-accum chunks) at 18..21.  Engine assignment per (tile, fold)
# balances DVE (4x, cheapest) vs ACT vs GPSIMD.
NSLOT = 22
SL_N = 0
SL_R = 9
SL_R0 = 18
ACT_R8_TILES = {0, 3, 6, 9}                  # R8 on ACT for these tiles
GP_N7_TILES = {2, 5, 8, 10}                  # N7 on GPSIMD for these tiles

MM_CHUNK = 512
PCOLS = NT * 256            # m cols: (t, h, c128) -> t*256 + h*128 + c
SBS = [(0, 2), (2, 4), (4, 6), (6, 8), (8, NT)]     # super-blocks

_BUILD_CACHE = {}


def _patch_tile_drain():
    """walrus rejects drains with >1 sync wait; split the tile-exit drain."""
    if getattr(tile.TileContext, "_drain_patched", False):
        return

    def _drain_and_barrier(self, tick_clock, wait_clock):
        drain_inst = self.nc.sync.drain()
        wait_clock.add_sem_waits(
            drain_inst.ins, ScopedClock({None: tick_clock.global_clock})
        )
        si = drain_inst.ins.sync_info
        if si is not None and len(si.on_wait) > 1:
            waits = list(si.on_wait)
            ups = list(si.on_update)
            drain_inst.ins.sync_info = mybir.SyncInfo(on_wait=waits[:1], on_update=[])
            last = drain_inst
            for i in range(1, len(waits)):
                last = self.nc.sync.drain()
                last.ins.sync_info = mybir.SyncInfo(on_wait=waits[i:i + 1], on_update=[])
            if ups:
                lw = list(last.ins.sync_info.on_wait) if last.ins.sync_info else []
                last.ins.sync_info = mybir.SyncInfo(on_wait=lw, on_update=ups)
        self.nc.all_engine_barrier()
        assert self.sems is not None
        popped = self.nc._tile_sem_poison_stack.pop()
        assert popped is self._sem_poison
        self.nc.clear_and_free_semaphores(list(self.sems.allocated().values()))
        self.nc.all_engine_barrier()

    tile.TileContext._drain_and_barrier = _drain_and_barrier
    tile.TileContext._drain_patched = True


def build_nc():
    _patch_tile_drain()
    nc = bacc.Bacc()

    # register threshold constants (ACT bias operands) as const APs
    for v in (float(-THR[8]), float(-THR[9])):
        if (F32, v) not in nc.const_aps.aps:
            tns = nc.alloc_sbuf_tensor(f"const-thr-{v}", [128, 1], F32)
            nc.gpsimd.memset(tns.ap(), v)
            nc.const_aps.aps[(F32, v)] = tns.ap()
    nc.all_engine_barrier()

    lg = nc.declare_dram_parameter("lg", [C, NPIX], F32, isOutput=False)
    zpad = nc.declare_dram_parameter("zpad", [C, TILE_F], F32, isOutput=False)
    bdq = nc.declare_dram_parameter("bdq", [P, 16 * 96], BF16, isOutput=False)
    bcneg = nc.declare_dram_parameter("bcneg", [G, P], BF16, isOutput=False)
    ident = nc.declare_dram_parameter("ident", [P, P], F32, isOutput=False)
    folds_out = nc.declare_dram_parameter("folds", [P, NT * NSLOT], F32, isOutput=True)
    m_out = nc.declare_dram_parameter("mlog", [96, PCOLS], F32, isOutput=True)

    with tile.TileContext(nc) as tc:
        with (
            tc.tile_pool(name="const", bufs=1) as constp,
            tc.tile_pool(name="lt", bufs=5) as lp,
            tc.tile_pool(name="et", bufs=2) as ep,
            tc.tile_pool(name="pt", bufs=3) as pp,
            tc.tile_pool(name="m6", bufs=2) as mp,
            tc.tile_pool(name="acc", bufs=1) as accp,
            tc.tile_pool(name="zpsum", bufs=2, space="PSUM") as zp,
            tc.tile_pool(name="dpsum", bufs=3, space="PSUM") as dp,
        ):
            bdq_sb = constp.tile([P, 16 * 96], BF16)
            nc.sync.dma_start(out=bdq_sb[:], in_=bdq[:])
            bc_sb = constp.tile([G, P], BF16)
            nc.sync.dma_start(out=bc_sb[:], in_=bcneg[:])
            id_sb = constp.tile([P, P], F32)
            nc.sync.dma_start(out=id_sb[:], in_=ident[:])

            foldacc = accp.tile([P, NT * NSLOT], F32)
            zbuf = accp.tile([96, PCOLS], F32)    # Z, then ln(Z) in place
            mhi = accp.tile([96, PCOLS], BF16)
            mlo = accp.tile([96, PCOLS], BF16)
            scr_dve = accp.tile([P, TILE_F], BF16)
            scr_act = accp.tile([P, TILE_F], BF16)
            scr_gp = accp.tile([P, TILE_F], BF16)

            lts = {}

            def phase_a(sb0, sb1):
                for t in range(sb0, sb1):
                    lt = lp.tile([P, TILE_F], F32)
                    lts[t] = lt
                    ng = G if t < PAD_TILE0 else G - 1
                    base = lg[:, t * TILE_F:(t + 1) * TILE_F]
                    src3 = bass_rust.AP(
                        tensor=base.tensor, offset=base.offset,
                        ap=[[F, ng]] + list(base.ap))
                    nc.sync.dma_start(out=lt[0:C * ng, :], in_=src3)
                    if ng < G:
                        nc.sync.dma_start(out=lt[C * 5:P, :], in_=zpad[:])

                    et = ep.tile([P, TILE_F], BF16)
                    nc.scalar.activation(et[:], lt[:], AF.Exp)

                    # Z[(g, qq), h*128 + c] for the whole tile in one PSUM
                    # tile [96, 256]; 16 accumulated matmuls per half
                    zps = zp.tile([96, 256], F32)
                    for h in range(2):
                        for qq in range(16):
                            c0 = h * 2048 + qq * 128
                            nc.tensor.matmul(
                                zps[:, h * 128:(h + 1) * 128],
                                bdq_sb[:, qq * 96:(qq + 1) * 96],
                                et[:, c0:c0 + 128],
                                start=(qq == 0), stop=(qq == 15),
                            )
                    nc.scalar.activation(
                        zbuf[:, t * 256:(t + 1) * 256], zps[:], AF.Copy,
                    )

            def phase_b(sb0, sb1):
                sl = slice(sb0 * 256, sb1 * 256)
                nc.scalar.activation(zbuf[:, sl], zbuf[:, sl], AF.Ln)
                nc.sync.dma_start(out=m_out[:, sl], in_=zbuf[:, sl])
                nc.scalar.activation(mhi[:, sl], zbuf[:, sl], AF.Copy)
                nc.gpsimd.tensor_sub(mlo[:, sl], zbuf[:, sl], mhi[:, sl])

            def phase_c(sb0, sb1):
                for t in range(sb0, sb1):
                    lt = lts.pop(t)
                    # m6: [6, 2*4096] bf16; col = s*4096 + qq*256 + h*128 + c
                    # (row-major unfold of mhi/mlo rows; the matmul moving
                    # operand for a (qt, h) 512-chunk is the strided AP
                    # [4 x 128 @ stride 256] at offset qt*1024 + h*128)
                    m6 = mp.tile([G, 2 * TILE_F], BF16)
                    msl = slice(t * 256, (t + 1) * 256)
                    nc.sync.dma_start(out=m6[:, 0:TILE_F], in_=mhi[:, msl])
                    nc.sync.dma_start(out=m6[:, TILE_F:2 * TILE_F],
                                      in_=mlo[:, msl])
                    m6ap = m6[:]

                    def m6_mov(off):
                        return bass_rust.AP(
                            tensor=m6ap.tensor, offset=m6ap.offset + off,
                            ap=[list(m6ap.ap[0]), [256, 4], [1, 128]])

                    pt = pp.tile([P, TILE_F], BF16)
                    base_sl = t * NSLOT
                    for hb in range(4):      # four [114, 1024] PSUM blocks
                        dps = dp.tile([P, 1024], F32)
                        for q in range(2):
                            n0 = hb * 1024 + q * MM_CHUNK
                            h, qt = n0 // 2048, (n0 % 2048) // MM_CHUNK
                            mcol = qt * 1024 + h * 128
                            out_sl = dps[:, q * MM_CHUNK:(q + 1) * MM_CHUNK]
                            nc.tensor.matmul(
                                out_sl, bc_sb[:], m6_mov(mcol),
                                start=True, stop=False,
                            )
                            nc.tensor.matmul(
                                out_sl, bc_sb[:], m6_mov(TILE_F + mcol),
                                start=False, stop=False,
                            )
                            nc.tensor.matmul(
                                out_sl, id_sb[:].bitcast(F32R),
                                lt[:, n0:n0 + MM_CHUNK].bitcast(F32R),
                                start=False, stop=True,
                            )
                        nc.scalar.activation(
                            pt[:, hb * 1024:(hb + 1) * 1024], dps[:], AF.Exp,
                            accum_out=foldacc[:, base_sl + SL_R0 + hb:
                                              base_sl + SL_R0 + hb + 1],
                        )

                    # ---- folds over bf16 p ----
                    gp_counts = [8, 9] + ([7] if t in GP_N7_TILES else [])
                    act_confs = [9] + ([8] if t in ACT_R8_TILES else [])
                    for i in range(1, 10):   # counts
                        col = foldacc[:, base_sl + SL_N + i - 1:
                                      base_sl + SL_N + i]
                        if i in gp_counts:
                            nc.gpsimd.tensor_scalar(
                                scr_gp[:], pt[:], float(THR[i]), None,
                                ALU.is_gt, ALU.add, accum_out=col,
                            )
                        else:
                            nc.vector.tensor_scalar(
                                scr_dve[:], pt[:], float(THR[i]), None,
                                ALU.is_gt, ALU.add, accum_out=col,
                            )
                    for i in range(1, 10):   # conf sums
                        col = foldacc[:, base_sl + SL_R + i - 1:
                                      base_sl + SL_R + i]
                        if i in act_confs:
                            nc.scalar.activation(
                                scr_act[:], pt[:], AF.Relu,
                                bias=-float(THR[i]), accum_out=col,
                            )
                        else:
                            nc.vector.tensor_scalar(
                                scr_dve[:], pt[:], float(THR[i]),
                                float(THR[i]), ALU.max, ALU.subtract,
                                accum_out=col,
                            )

            # software-pipelined phase order: A0 B0 A1 C0 B1 A2 C1 ...
            phase_a(*SBS[0])
            phase_b(*SBS[0])
            for k in range(1, len(SBS)):
                phase_a(*SBS[k])
                phase_c(*SBS[k - 1])
                phase_b(*SBS[k])
            phase_c(*SBS[-1])

            # ---- end phase ----
            nc.sync.dma_start(out=folds_out[:], in_=foldacc[:])

    nc.finalize()
    return nc


def _make_consts():
    # bdq: 16 stationaries [114, 96]; block qq maps class-group g of
    # moving chunk qq*128 to output row (g, qq)
    bdq = np.zeros((P, 16 * 96), np.float32)
    for qq in range(16):
        for g in range(G):
            bdq[C * g:C * (g + 1), qq * 96 + g * 16 + qq] = 1.0
    bc = np.zeros((G, P), np.float32)
    for g in range(G):
        bc[g, C * g:C * (g + 1)] = -1.0
    return bdq, bc


def _shard_host(output: np.ndarray, target: np.ndarray):
    o = np.ascontiguousarray(output[0])          # [19, 1024, 2048]
    bdq, bc = _make_consts()
    zp = np.full((C, TILE_F), PADLO, np.float32)
    zp[0, :] = PADHI

    in_maps = []
    for core in range(NCORES):
        r0 = core * ROWS
        lgc = np.ascontiguousarray(o[:, r0:r0 + ROWS, :].reshape(C, NPIX))
        in_maps.append({
            "lg": lgc, "bdq": bdq, "bcneg": bc,
            "ident": np.eye(P, dtype=np.float32),
            "zpad": zp,
        })
    return in_maps


def _m_to_flat(m: np.ndarray) -> np.ndarray:
    """[96, PCOLS] (g,qq)-major -> core-flat [G*F] (incl pad)."""
    return (m.reshape(G, 16, NT, 2, 128).transpose(0, 2, 3, 1, 4).reshape(-1))


def _decode_and_loss(results, output: np.ndarray, target: np.ndarray):
    conf = np.zeros((C, NB), np.float64)
    cnt = np.zeros((C, NB), np.float64)
    acc = np.zeros((C, NB), np.float64)

    o = output[0]
    lstar_full = np.take_along_axis(o, target[0][None].astype(np.int64), axis=0)[0]

    for core in range(NCORES):
        folds = results[core]["folds"].astype(np.float64)
        folds = folds.reshape(P, NT, NSLOT).sum(axis=1)      # [114, 22]
        folds = folds.reshape(G, C, NSLOT).sum(axis=0)       # [C, 22]
        Ni = np.concatenate(
            [np.full((C, 1), float(NPIX)),
             folds[:, SL_N:SL_N + 9]], axis=1)               # [C, 10]
        R = np.concatenate(
            [folds[:, SL_R0:SL_R0 + 4].sum(axis=1, keepdims=True),
             folds[:, SL_R:SL_R + 9]], axis=1)               # [C, 10]

        # pad pixels land on class 0 with p = 1.0 exactly: remove them
        Ni[0, 1:] -= NPAD
        tgrid = np.arange(10, dtype=np.float64) / 10.0
        R[0, :] -= NPAD * (1.0 - tgrid)

        S = R + tgrid[None, :] * Ni              # S_i = sum p * [p > t_i]
        Snext = np.concatenate([S[:, 1:], np.zeros((C, 1))], axis=1)
        Nnext = np.concatenate([Ni[:, 1:], np.zeros((C, 1))], axis=1)
        conf += S - Snext
        cnt += Ni - Nnext

        # accuracy histogram from host-side p* = exp(l* - m)
        r0 = core * ROWS
        m = _m_to_flat(results[core]["mlog"])[:NPIX]
        ls = lstar_full[r0:r0 + ROWS, :].reshape(-1)
        ps = np.exp(ls - m).astype(np.float32)
        y = target[0, r0:r0 + ROWS, :].reshape(-1)
        b = np.clip(np.ceil(ps * np.float32(10.0)).astype(np.int32) - 1, 0, NB - 1)
        acc += np.bincount(y * NB + b, minlength=C * NB).reshape(C, NB)

    EPS = 1e-13
    avg_acc = acc / (cnt + EPS)
    avg_conf = conf / (cnt + EPS)
    loss = np.sum((avg_acc - avg_conf) ** 2 * (cnt / cnt.sum()))
    return np.float32(loss), (conf, cnt, acc)


def kernel(output: np.ndarray, target: np.ndarray) -> np.ndarray:
    output = np.asarray(output, np.float32)
    target = np.asarray(target, np.int32)
    if "nc" not in _BUILD_CACHE:
        _BUILD_CACHE["nc"] = build_nc()
    nc = _BUILD_CACHE["nc"]
    in_maps = _shard_host(output, target)
    res = run_bass_kernel_spmd(nc, in_maps, list(range(NCORES)))
    loss, _ = _decode_and_loss(res.results, output, target)
    return np.float32(loss)
